# revision 1
# baseline (speedup 1.0000x reference)
"""Trainium2 Bass kernel: Conv2d(1->64, k=7, valid) on data [32,1,224,224] f32.

Data-parallel over batch (4 images per core on 8 cores).  Per core:
im2col matmul in fp16 (K=49 taps, M=64 out-channels), PSUM fp32, fp32 out.

Layout/pipeline (per core, 32 row-block "tiles" of 28 output rows):
  - host: fp16 cast; for each tile, SEVEN copies of its 34-row source
    block, copy ky pre-shifted down by ky rows.  Copies of one tile sit at
    partitions base+4*ky (7 distinct AXI ports); even tiles use the lower
    partition half / even ports, odd tiles the upper half / odd ports.
  - im2col: ONE SWDGE DMA per tile (3-dim AP): src dim0 walks the 7 slab
    copies, dim1 the 7 kx shifts (overlapping reads), dim2 a contiguous
    28*224-col run.  dst = [49, 6272] fp16 at partition base 0 (even
    tiles) or 64 (odd tiles).
  - matmul: pairs (even tile, odd tile): lhsT = W^T [49,64] fp16 at row
    base 0/64, out -> psum[0:64]/[64:128] of one bank.  Alternating row
    groups lets LDWEIGHTS overlap in-flight matmuls.
  - copy: psum [128,448] -> ob tile full width, DVE/ACT alternating.
  - out: one DMA per tile [64ch, 28*224 f32]; even tiles on the sync
    HWDGE ring, odd on scalar.  Cols 218..223 are garbage (kx wrap) and
    are sliced off on the host, as are rows >= 218.
"""

import numpy as np

B = 32            # full batch
OC = 64           # out channels
KS = 7            # kernel size
H = 224           # input H=W
OH = 218          # valid output rows/cols
OW = 224          # computed output width (incl 6 garbage cols)
NCORES = 8
IPC = B // NCORES  # images per core

BLK = 28          # output rows per tile
NBLK = 8          # tiles per image
SRC_ROWS = 34     # rows stored per slab copy
SLAB = SRC_ROWS * H + 8   # 7624 fp16 elements per slab
NTILES = IPC * NBLK       # 32 tiles per core
NPAIRS = NTILES // 2
NCOLS = BLK * OW          # 6272 im2col columns per tile
NMM = NCOLS // 448        # 14 matmuls per tile

# slab-group bases: tile t -> 7 slabs at partitions base+4*ky, where
# base = (64 if t odd) + BASES[(t//2) % 8], free slot (t//2) // 8.
BASES = [0, 1, 2, 3, 28, 29, 30, 31]

_CACHE = {}


def _tile_src(t):
    q = t // 2
    base = BASES[q % 8] + (64 if (t % 2) else 0)
    return base, q // 8  # partition base, slot


def _build():
    import concourse.bass as bass
    import concourse.mybir as mybir
    import concourse.tile as tile
    from concourse import bacc

    nc = bacc.Bacc("TRN2", target_bir_lowering=False, debug=False)

    xb = nc.dram_tensor("xb", [2, 128, SLAB], mybir.dt.float16,
                        kind="ExternalInput")
    wT = nc.dram_tensor("wT", [KS * KS, OC], mybir.dt.float16,
                        kind="ExternalInput")
    out = nc.dram_tensor("out", [IPC, OC, OH, OW], mybir.dt.float32,
                         kind="ExternalOutput")

    with tile.TileContext(nc) as tc:
        with (
            tc.tile_pool(name="src", bufs=1) as src_pool,
            tc.tile_pool(name="wp", bufs=1) as w_pool,
            tc.tile_pool(name="i2c", bufs=8) as i2c_pool,
            tc.tile_pool(name="ob", bufs=3) as ob_pool,
            tc.tile_pool(name="ps", bufs=8, space="PSUM") as ps_pool,
        ):
            srct = src_pool.tile([128, 2 * SLAB], mybir.dt.float16)
            wt = w_pool.tile([128, OC], mybir.dt.float16)

            p_stride = srct.ap[0][0]  # partition pitch in elements

            nc.sync.dma_start(out=wt[0:49, :], in_=wT[:, :])
            nc.sync.dma_start(out=wt[64:113, :], in_=wT[:, :])
            for slot in range(2):
                nc.sync.dma_start(
                    out=srct[:, slot * SLAB:(slot + 1) * SLAB],
                    in_=xb[slot, :, :])

            # software-pipelined emission: issue im2col DMAs PREFETCH pairs
            # ahead of the compute stream so the POOL engine's in-order
            # instruction stream never blocks descriptor emission on a
            # downstream dependency.
            PREFETCH = 7
            i2c_tiles = {}

            def issue_i2c(q):
                i2c = i2c_pool.tile([128, NCOLS], mybir.dt.float16,
                                    tag="i2c", name=f"i2c{q}")
                for half in range(2):
                    t = 2 * q + half
                    base, slot = _tile_src(t)
                    src = bass.AP(
                        tensor=srct.tensor,
                        offset=srct.offset + base * p_stride + slot * SLAB,
                        ap=[[4 * p_stride, KS], [1, KS], [1, NCOLS]],
                    )
                    b0 = 64 * half
                    nc.gpsimd.dma_start(
                        out=i2c[b0:b0 + KS * KS, :], in_=src)
                i2c_tiles[q] = i2c

            for q in range(min(PREFETCH, NPAIRS)):
                issue_i2c(q)

            for q in range(NPAIRS):
                if q + PREFETCH < NPAIRS:
                    issue_i2c(q + PREFETCH)
                i2c = i2c_tiles.pop(q)

                ob = ob_pool.tile([128, NCOLS], mybir.dt.float16, tag="ob")
                for j in range(NMM):
                    ps = ps_pool.tile([128, 448], mybir.dt.float32, tag="ps")
                    nc.tensor.matmul(
                        ps[0:OC, :], wt[0:49, :],
                        i2c[0:49, 448 * j: 448 * (j + 1)],
                        start=True, stop=True)
                    nc.tensor.matmul(
                        ps[OC:128, :], wt[64:113, :],
                        i2c[64:113, 448 * j: 448 * (j + 1)],
                        start=True, stop=True)
                    if j % 2 == 0:
                        nc.vector.tensor_copy(
                            ob[:, 448 * j: 448 * (j + 1)], ps[:, :])
                    else:
                        nc.scalar.copy(
                            ob[:, 448 * j: 448 * (j + 1)], ps[:, :])

                # fp16 -> fp32 cast during the store; only SWDGE casts
                for half in range(2):
                    t = 2 * q + half
                    img, blk = divmod(t, NBLK)
                    r0 = BLK * blk
                    nrows = min(BLK, OH - r0)
                    nc.gpsimd.dma_start(
                        out=out[img, :, r0: r0 + nrows, :],
                        in_=ob[64 * half: 64 * half + OC, : nrows * OW])

    nc.compile()
    return nc


def _prep_inputs(data, weight):
    d16 = np.ascontiguousarray(data.reshape(B, H, H)).astype(np.float16)
    dpad = np.zeros((B, 256, H), dtype=np.float16)
    dpad[:, :H, :] = d16
    wt = np.ascontiguousarray(
        weight.reshape(OC, KS * KS).T).astype(np.float16)

    in_maps = []
    for c in range(NCORES):
        xb = np.zeros((2, 128, SLAB), dtype=np.float16)
        for t in range(NTILES):
            img, blk = divmod(t, NBLK)
            gimg = c * IPC + img
            base, slot = _tile_src(t)
            for ky in range(KS):
                r0 = BLK * blk + ky
                xb[slot, base + 4 * ky, : SRC_ROWS * H] = \
                    dpad[gimg, r0: r0 + SRC_ROWS, :].ravel()
        in_maps.append({"xb": xb, "wT": wt})
    return in_maps


def kernel(data, weight):
    from concourse.bass_utils import run_bass_kernel_spmd

    if "nc" not in _CACHE:
        _CACHE["nc"] = _build()
    nc = _CACHE["nc"]

    in_maps = _prep_inputs(np.asarray(data), np.asarray(weight))
    res = run_bass_kernel_spmd(nc, in_maps, core_ids=list(range(NCORES)))
    outs = [r["out"] for r in res.results]
    full = np.concatenate(outs, axis=0)  # [32, 64, 218, 224]
    return np.ascontiguousarray(full[:, :, :, :OH]).astype(np.float32)



# revision 11
# speedup vs baseline: 1.4469x; 1.4469x over previous
"""Trainium2 Bass kernel: Conv2d(1->64, k=7, valid) on data [32,1,224,224] f32.

Data-parallel over batch (4 images per core on 8 cores).  Per core the conv
is an im2col matmul with THREE structural tricks vs the naive version:

1. Paired output rows: each matmul column computes 2 output rows x 64
   channels (M=128).  Column (pair p, x) needs taps in[r0+2p+ky', x+kx]
   for ky' in 0..7, kx in 0..6.
2. kx-split accumulation: only kx 0..3 is materialized (K=32 taps = one
   PE row strip).  Two accumulating matmuls per PSUM chunk: A with
   weights for kx 0..3, B with weights for kx 4..6 reading the SAME
   im2col tile at a +4 element offset (the kx shift is an AP offset).
3. Row-strip packing: 4 tiles run concurrently in the PE array at strips
   0..3 (tile_position=(32s,0)), each K=32, M=128.

im2col is ONE SWDGE DMA per (tile, row-parity): source is a per-tile slab
of 18 even (or odd) rows stored contiguously on a single partition, read
with overlapping APs [[224,4],[1,4],[1,3144]] -> 16 dst partitions, fully
contiguous 6.3KB runs.  Taps at partition 32s + par*16 + ky2*4 + kx.

PSUM [128,1024] 2-bank tiles; chunks of 448 cols at offsets 0/512.
DVE/ACT copies compact the 224-wide pairs to 218 valid cols in ob, so the
fp16 output stores are fully contiguous (6.1KB descriptors) on HWDGE.
Output DRAM layout [IPC, 64, 2(parity), 109, 218]; host re-interleaves
rows and casts to fp32.
"""

import numpy as np

B = 32            # full batch
OC = 64           # out channels
KS = 7            # kernel size
H = 224           # input H=W
OH = 218          # valid output rows/cols
NCORES = 8
IPC = B // NCORES  # images per core

BLK = 28          # output rows per tile
NBLK = 8          # tiles per image
NTILES = IPC * NBLK       # 32 tiles per core
NGRP = NTILES // 4        # 8 groups of 4 concurrent tiles
SLAB_ROWS = 15            # rows per (parity, ky2) slab copy
SLAB = SLAB_ROWS * H      # 3360
SLABP = SLAB + 8          # padded slab pitch
SLOTW = 2 * SLABP         # two slab slots per partition
NCOL = 3144               # im2col cols per tile (14*224 + 8 pad)
CW = 448                  # matmul chunk width (2 pairs x 224)
NPAIRS = 14               # row pairs per tile
OBW = NPAIRS * OH         # 3052 compact ob cols
NPIMG = 109               # row pairs per image (218/2)

_CACHE = {}


def _slab_pos(t, par):
    # slab copies for (tile, parity, ky2) sit on 4 consecutive partitions
    # (ky2 = partition step, required: SBUF AP dim0 must step partitions).
    k = 8 * t + 4 * par
    return k % 128, k // 128  # partition base, slot


def _build():
    import concourse.bass as bass
    import concourse.mybir as mybir
    import concourse.tile as tile
    from concourse import bacc

    nc = bacc.Bacc("TRN2", target_bir_lowering=False, debug=False)

    xb = nc.dram_tensor("xb", [128, SLOTW], mybir.dt.float16,
                        kind="ExternalInput")
    wa = nc.dram_tensor("wa", [128, 128], mybir.dt.float16,
                        kind="ExternalInput")
    wb = nc.dram_tensor("wb", [128, 128], mybir.dt.float16,
                        kind="ExternalInput")
    out = nc.dram_tensor("out", [IPC, OC, 2, NPIMG, OH], mybir.dt.float16,
                         kind="ExternalOutput")

    with tile.TileContext(nc) as tc:
        with (
            tc.tile_pool(name="src", bufs=1) as src_pool,
            tc.tile_pool(name="wp", bufs=1) as w_pool,
            tc.tile_pool(name="i2c", bufs=4) as i2c_pool,
            tc.tile_pool(name="ob", bufs=6) as ob_pool,
            tc.tile_pool(name="ps2", bufs=3, space="PSUM") as ps2_pool,
            tc.tile_pool(name="ps1", bufs=2, space="PSUM") as ps1_pool,
        ):
            oap = out.ap()
            srct = src_pool.tile([128, SLOTW], mybir.dt.float16)
            wta = w_pool.tile([128, 128], mybir.dt.float16)
            wtb = w_pool.tile([128, 128], mybir.dt.float16)

            p_stride = srct.ap[0][0]  # partition pitch in elements

            nc.sync.dma_start(out=wta[:, :], in_=wa[:, :])
            nc.sync.dma_start(out=wtb[:, :], in_=wb[:, :])
            nc.sync.dma_start(out=srct[:, :], in_=xb[:, :])

            # software-pipelined emission: issue im2col DMAs PREFETCH
            # groups ahead so POOL's in-order stream never stalls emission.
            PREFETCH = 2
            i2c_tiles = {}

            def issue_i2c(g):
                i2c = i2c_pool.tile([128, NCOL], mybir.dt.float16,
                                    tag="i2c", name=f"i2c{g}")
                for s in range(4):
                    t = 4 * g + s
                    for par in range(2):
                        p0, slot = _slab_pos(t, par)
                        src = bass.AP(
                            tensor=srct.tensor,
                            offset=srct.offset + p0 * p_stride
                            + slot * SLABP,
                            ap=[[p_stride, 4], [1, 4], [1, NCOL]],
                        )
                        b0 = 32 * s + 16 * par
                        nc.gpsimd.dma_start(
                            out=i2c[b0:b0 + 16, :], in_=src)
                i2c_tiles[g] = i2c

            for g in range(min(PREFETCH, NGRP)):
                issue_i2c(g)

            for g in range(NGRP):
                if g + PREFETCH < NGRP:
                    issue_i2c(g + PREFETCH)
                i2c = i2c_tiles.pop(g)

                for s in range(4):
                    t = 4 * g + s
                    img, blk = divmod(t, NBLK)
                    npair = NPAIRS if blk < NBLK - 1 else NPIMG - NPAIRS * blk
                    ob = ob_pool.tile([128, OBW + 4], mybir.dt.float16,
                                      tag="ob")
                    rhs = i2c[32 * s:32 * s + 32, :]
                    lha = wta[32 * s:32 * s + 32, :]
                    lhb = wtb[32 * s:32 * s + 32, :]
                    tp = (32 * s, 0)

                    for pc in range(3):  # chunk pairs (0,1),(2,3),(4,5)
                        ps = ps2_pool.tile([128, 1024], mybir.dt.float32,
                                           tag="ps2")
                        for h in range(2):
                            c0 = (2 * pc + h) * CW
                            nc.tensor.matmul(
                                ps[:, 512 * h:512 * h + CW], lha,
                                rhs[:, c0:c0 + CW],
                                start=True, stop=False, tile_position=tp)
                        for h in range(2):
                            c0 = (2 * pc + h) * CW + 4
                            nc.tensor.matmul(
                                ps[:, 512 * h:512 * h + CW], lhb,
                                rhs[:, c0:c0 + CW],
                                start=False, stop=True, tile_position=tp)
                        # compact copy: drop the 6 garbage cols per 224
                        csrc = bass.AP(
                            tensor=ps.tensor, offset=ps.offset,
                            ap=[list(ps.ap[0]), [512, 2], [224, 2], [1, OH]])
                        cdst = ob[:, 4 * pc * OH: (4 * pc + 4) * OH]
                        if pc % 2 == 0:
                            nc.vector.tensor_copy(cdst, csrc)
                        else:
                            nc.scalar.copy(cdst, csrc)

                    # 7th chunk (pairs 12,13) in a 1-bank psum tile
                    ps = ps1_pool.tile([128, 512], mybir.dt.float32,
                                       tag="ps1")
                    c0 = 6 * CW
                    nc.tensor.matmul(ps[:, 0:CW], lha, rhs[:, c0:c0 + CW],
                                     start=True, stop=False, tile_position=tp)
                    nc.tensor.matmul(ps[:, 0:CW], lhb,
                                     rhs[:, c0 + 4:c0 + 4 + CW],
                                     start=False, stop=True, tile_position=tp)
                    csrc = bass.AP(
                        tensor=ps.tensor, offset=ps.offset,
                        ap=[list(ps.ap[0]), [224, 2], [1, OH]])
                    nc.scalar.copy(ob[:, 12 * OH:14 * OH], csrc)

                    # fp16 stores, fully contiguous runs, on the 2 HWDGE rings
                    for par in range(2):
                        dst = bass.AP(
                            tensor=oap.tensor,
                            offset=oap.offset
                            + ((img * OC * 2 + par) * NPIMG
                               + NPAIRS * blk) * OH,
                            ap=[[2 * NPIMG * OH, OC], [1, npair * OH]],
                        )
                        eng = nc.sync if par == 0 else nc.scalar
                        eng.dma_start(
                            out=dst,
                            in_=ob[64 * par:64 * par + OC, :npair * OH])

    nc.compile()
    return nc


def _prep_inputs(data, weight):
    d16 = np.asarray(data).reshape(B, H, H).astype(np.float16)
    dpad = np.zeros((B, 236, H), dtype=np.float16)
    dpad[:, :H, :] = d16

    w = np.asarray(weight).reshape(OC, KS, KS).astype(np.float32)
    wa32 = np.zeros((32, 128), dtype=np.float32)
    wb32 = np.zeros((32, 128), dtype=np.float32)
    for par in range(2):
        for ky2 in range(4):
            kyp = 2 * ky2 + par
            for c in range(4):
                idx = par * 16 + ky2 * 4 + c
                if kyp <= 6:
                    wa32[idx, :OC] = w[:, kyp, c]
                    if c <= 2:
                        wb32[idx, :OC] = w[:, kyp, c + 4]
                if 1 <= kyp <= 7:
                    wa32[idx, OC:] = w[:, kyp - 1, c]
                    if c <= 2:
                        wb32[idx, OC:] = w[:, kyp - 1, c + 4]
    wa = np.tile(wa32, (4, 1)).astype(np.float16)
    wb = np.tile(wb32, (4, 1)).astype(np.float16)

    in_maps = []
    for core in range(NCORES):
        xb = np.zeros((128, SLOTW), dtype=np.float16)
        for t in range(NTILES):
            img, blk = divmod(t, NBLK)
            gimg = core * IPC + img
            r0 = BLK * blk
            for par in range(2):
                p0, slot = _slab_pos(t, par)
                for ky2 in range(4):
                    rs = r0 + par + 2 * ky2
                    xb[p0 + ky2, slot * SLABP: slot * SLABP + SLAB] = \
                        dpad[gimg, rs: rs + 2 * SLAB_ROWS: 2, :].ravel()
        in_maps.append({"xb": xb, "wa": wa, "wb": wb})
    return in_maps


def kernel(data, weight):
    from concourse.bass_utils import run_bass_kernel_spmd

    if "nc" not in _CACHE:
        _CACHE["nc"] = _build()
    nc = _CACHE["nc"]

    in_maps = _prep_inputs(data, weight)
    res = run_bass_kernel_spmd(nc, in_maps, core_ids=list(range(NCORES)))
    outs = [r["out"] for r in res.results]  # [IPC, 64, 2, 109, 218] each
    full = np.concatenate(outs, axis=0)     # [32, 64, 2, 109, 218]
    final = np.empty((B, OC, OH, OH), dtype=np.float32)
    final[:, :, 0::2, :] = full[:, :, 0]
    final[:, :, 1::2, :] = full[:, :, 1]
    return final
